# revision 2
# baseline (speedup 1.0000x reference)
"""DINOv2 LoRA featurizer histogram-binning kernel for TRN2 (8 NeuronCores).

Reference computation (per sample):
  x: [37, 37, 384] -> bx = x^T [384, 37, 37]
  pool0 = bx, pool1 = AvgPool2d(3, stride 1, pad 1, count_include_pad=False)
  17 bins = border-clamped shifts of pool0 (9 bins, offsets +-1) and
  pool1 (8 bins, offsets +-3); bins 17..28 of 29 are zero.
  out = [29*384, 37, 37] with channel c = bin*384 + feature.

Sharding: pure data parallel, sample b -> core b (B == 8 == n_cores).

The kernel is store-bandwidth bound (17 bins x 2.1 MB per core). This
version halves the store stream by writing fp16 (one final rounding,
rel err <= 2^-11, far inside the 2e-2 gate) and keeps only the 17 real
bins in DRAM; the 12 zero bins are assembled host-side as zeros.

Device strategy (per core), built so the store-DMA stream (~18 MB fp16)
is the only critical path:
  - channels on partitions (3 tiles of 128), spatial flattened in free dim
  - x is uploaded as fp16 and DMA'd straight into the dx=0 pool0 plane,
    so k=0 bins are exactly fp16(x) (one rounding from the reference)
  - for each (pool k, dx) a column-pre-shifted, row-replicated-padded plane
    R[k][dxi][t]; every bin is then a CONTIGUOUS row-window of one plane, so
    bin stores are pure DMAs grouped per dy (2-3 bins x 1-3 ctiles)
  - dx=+-1 planes are flat-shifted whole-plane fp16 copies (column shift ==
    +-1 element offset in the flat layout; row-boundary wrap errors land
    exactly in the clamped columns, fixed by one strided edge-column copy)
  - pool1 separable 3x3 SUM runs in f32 (T, Sp tiles) with FLAT shifted
    adds + tiny edge fixes; the count_include_pad=False normalization
    factorizes (cnt = rowcnt[i]*colcnt[j], each in {2,3}), so ONE final
    tensor_scalar mul by 1/9 writes the fp16 plane directly and four tiny
    edge-region muls by 1.5 fix the border rows/columns
  - no stride-0 (broadcast) APs, no GpSimd data ops (both measured slow)
"""

import numpy as np

B = 8
W = 37          # spatial side
WW = W * W      # 1369
D = 384
P = 128
ST = D // P     # 3 channel tiles of 128
NBINS = 29
NWR = 17        # bins actually written (the rest are zero)
PAD0, PAD1 = 1, 3
R0ROWS = W + 2 * PAD0             # 39
R1ROWS = W + 2 * PAD1             # 43
R0F = R0ROWS * W                  # 1443 flat elems per plane
R1F = R1ROWS * W                  # 1591

_CACHE = {}


def _build_nc():
    import concourse.bass as bass  # noqa: F401
    import concourse.tile as tile
    from concourse import bacc, mybir
    from contextlib import ExitStack

    f32 = mybir.dt.float32
    f16 = mybir.dt.float16
    nc = bacc.Bacc("TRN2", target_bir_lowering=False, debug=False)

    xt = nc.declare_dram_parameter("xt", [ST, P, WW], f16, isOutput=False)
    out = nc.declare_dram_parameter("out", [NWR, ST, P, WW], f16, isOutput=True)

    with tile.TileContext(nc) as tc, ExitStack() as ctx:
        perm = ctx.enter_context(tc.tile_pool(name="perm", bufs=1))
        tmp = ctx.enter_context(tc.tile_pool(name="tmp", bufs=2))

        # R0: [dxi, t, 39, 37] (pad 1, dx in {-1,0,+1})
        # R1: [dxi, t, 43, 37] (pad 3, dx in {-3,0,+3})
        R0 = perm.tile([P, 3, ST, R0ROWS, W], f16, name="R0")
        R1 = perm.tile([P, 3, ST, R1ROWS, W], f16, name="R1")

        # ---- load fp16 x into the dx=0 plane centers first (idle SWDGE
        # queue, so compute waits on other engines can never delay a load)
        for t in range(ST):
            nc.gpsimd.dma_start(
                R0[:, 1, t, PAD0 : PAD0 + W, :].rearrange("p a b -> p (a b)"),
                xt.ap()[t],
            )

        for t in range(ST):
            # pad rows of the dx=0 plane (replicate first/last x row)
            nc.vector.tensor_copy(R0[:, 1, t, 0, :], R0[:, 1, t, 1, :])
            nc.vector.tensor_copy(R0[:, 1, t, R0ROWS - 1, :], R0[:, 1, t, R0ROWS - 2, :])

            # ---- dx=+-1 planes: flat-shifted whole-plane copies + col fix
            # (pad rows propagate; wrap errors land in the clamped column) ----
            p0f = R0[:, 1, t].rearrange("p a b -> p (a b)")
            f = R0[:, 0, t].rearrange("p a b -> p (a b)")
            nc.scalar.copy(f[:, 1:R0F], p0f[:, 0 : R0F - 1])
            nc.scalar.copy(R0[:, 0, t, :, 0], R0[:, 1, t, :, 0])
            f = R0[:, 2, t].rearrange("p a b -> p (a b)")
            nc.vector.tensor_copy(f[:, 0 : R0F - 1], p0f[:, 1:R0F])
            nc.vector.tensor_copy(R0[:, 2, t, :, W - 1], R0[:, 1, t, :, W - 1])

            # ---- k=0 stores for this ctile (one DMA per dy group) ----
            for r_i, dy in enumerate((-1, 0, 1)):
                src = R0[:, :, t, PAD0 + dy : PAD0 + dy + W, :].rearrange(
                    "p x a b -> p x (a b)"
                )
                dst = out.ap()[3 * r_i : 3 * r_i + 3, t].transpose([1, 0, 2])
                nc.sync.dma_start(dst, src)

        for t in range(ST):
            Xc = R0[:, 1, t, PAD0 : PAD0 + W, :].rearrange("p a b -> p (a b)")
            Xc3 = R0[:, 1, t, PAD0 : PAD0 + W, :]

            # ---- column pass (f32 sums from fp16 x):
            # T[i,j] = sum_dx X[i, j+dx] (zero outside) ----
            T = tmp.tile([P, WW], f32, name="T", tag="T")
            T3 = T.rearrange("p (a b) -> p a b", a=W, b=W)
            nc.vector.tensor_add(T[:, 0 : WW - 1], Xc[:, 0 : WW - 1], Xc[:, 1:WW])
            nc.vector.tensor_copy(T[:, WW - 1 : WW], Xc[:, WW - 1 : WW])
            nc.vector.tensor_add(T[:, 1:WW], T[:, 1:WW], Xc[:, 0 : WW - 1])
            nc.vector.tensor_add(T3[:, :, 0], Xc3[:, :, 0], Xc3[:, :, 1])
            nc.vector.tensor_add(T3[:, :, W - 1], Xc3[:, :, W - 2], Xc3[:, :, W - 1])

            # ---- row pass into a padded f32 SUM plane (normalization is
            # applied in one shot by the final fp16-writing mul) ----
            Sp = tmp.tile([P, R1F], f32, name="Sp", tag="Sp")
            c0 = PAD1 * W                        # 111: center start
            nW = WW - W
            nc.vector.tensor_add(Sp[:, c0 : c0 + nW], T[:, 0:nW], T[:, W:WW])
            nc.vector.tensor_copy(Sp[:, c0 + nW : c0 + WW], T[:, nW:WW])
            nc.vector.tensor_add(Sp[:, c0 + W : c0 + WW], Sp[:, c0 + W : c0 + WW], T[:, 0:nW])
            # pad rows: replicate first/last center row (contiguous copies)
            for i in range(PAD1):
                nc.scalar.copy(Sp[:, i * W : (i + 1) * W], Sp[:, c0 : c0 + W])
                nc.scalar.copy(
                    Sp[:, (PAD1 + W + i) * W : (PAD1 + W + i + 1) * W],
                    Sp[:, (PAD1 + W - 1) * W : (PAD1 + W) * W],
                )

            # ---- normalize + round to fp16 in ONE pass: interior count is
            # 9; border rows/cols have count 2 (not 3) per axis -> x1.5 ----
            Pc = R1[:, 1, t]
            Pcf = Pc.rearrange("p a b -> p (a b)")
            nc.vector.tensor_scalar_mul(Pcf[:, :], Sp[:, :], 1.0 / 9.0)
            nc.vector.tensor_scalar_mul(Pcf[:, 0 : c0 + W], Pcf[:, 0 : c0 + W], 1.5)
            nc.vector.tensor_scalar_mul(
                Pcf[:, R1F - c0 - W : R1F], Pcf[:, R1F - c0 - W : R1F], 1.5
            )
            nc.vector.tensor_scalar_mul(Pc[:, :, 0], Pc[:, :, 0], 1.5)
            nc.vector.tensor_scalar_mul(Pc[:, :, W - 1], Pc[:, :, W - 1], 1.5)

            # ---- dx=+-3 planes: flat-shifted fp16 copies of the dx=0 plane ----
            f = R1[:, 0, t].rearrange("p a b -> p (a b)")
            nc.scalar.copy(f[:, 3:R1F], Pcf[:, 0 : R1F - 3])
            f = R1[:, 2, t].rearrange("p a b -> p (a b)")
            nc.vector.tensor_copy(f[:, 0 : R1F - 3], Pcf[:, 3:R1F])

        # edge-column fixes for R1 dx=+-3 planes (all t, all 43 rows)
        P1c = R1[:, 1]
        for c in range(PAD1):
            nc.scalar.copy(R1[:, 0, :, :, c], P1c[:, :, :, 0])
            nc.scalar.copy(R1[:, 2, :, :, W - 1 - c], P1c[:, :, :, W - 1])

        # ---- k=1 stores: bins grouped per dy (all ctiles, all dx at once) ----
        for dy, dxis, p0 in ((-3, None, 9), (0, 0, 12), (0, 2, 13), (3, None, 14)):
            lo = PAD1 + dy
            if dxis is None:
                src = R1[:, :, :, lo : lo + W, :].rearrange("p x t a b -> p (x t) (a b)")
                dst = out.ap()[p0 : p0 + 3].transpose([2, 0, 1, 3]).rearrange(
                    "p x t e -> p (x t) e"
                )
            else:
                src = R1[:, dxis, :, lo : lo + W, :].rearrange("p t a b -> p t (a b)")
                dst = out.ap()[p0].transpose([1, 0, 2])
            nc.sync.dma_start(dst, src)

    nc.compile()
    return nc


def get_nc():
    if "nc" not in _CACHE:
        _CACHE["nc"] = _build_nc()
    return _CACHE["nc"]


def make_in_maps(x: np.ndarray):
    x = np.ascontiguousarray(x, dtype=np.float32)
    assert x.shape == (B, W, W, D), x.shape
    maps = []
    for b in range(B):
        xtr = x[b].transpose(2, 0, 1).reshape(ST, P, WW).astype(np.float16)
        maps.append({"xt": np.ascontiguousarray(xtr)})
    return maps


def run(x: np.ndarray, **kw):
    from concourse.bass_utils import run_bass_kernel_spmd

    nc = get_nc()
    res = run_bass_kernel_spmd(nc, make_in_maps(x), core_ids=list(range(B)), **kw)
    outs = np.zeros((B, NBINS * D, W, W), np.float32)
    for b in range(B):
        ob = np.asarray(res.results[b]["out"], dtype=np.float32)
        outs[b, : NWR * D] = ob.reshape(NWR * D, W, W)
    return outs, res


def kernel(x: np.ndarray) -> np.ndarray:
    outs, _ = run(x)
    return outs


# revision 3
# speedup vs baseline: 1.0177x; 1.0177x over previous
"""DINOv2 LoRA featurizer histogram-binning kernel for TRN2 (8 NeuronCores).

Reference computation (per sample):
  x: [37, 37, 384] -> bx = x^T [384, 37, 37]
  pool0 = bx, pool1 = AvgPool2d(3, stride 1, pad 1, count_include_pad=False)
  17 bins = border-clamped shifts of pool0 (9 bins, offsets +-1) and
  pool1 (8 bins, offsets +-3); bins 17..28 of 29 are zero.
  out = [29*384, 37, 37] with channel c = bin*384 + feature.

Sharding: pure data parallel, sample b -> core b (B == 8 == n_cores).

The kernel is store-bandwidth bound (17 bins x 2.1 MB per core). This
version halves the store stream by writing fp16 (rel err ~4e-4, far
inside the 2e-2 gate) and keeps only the 17 real bins in DRAM; the 12
zero bins are assembled host-side as zeros.

Device strategy (per core), built so the store-DMA stream (~18 MB fp16)
is the only critical path:
  - channels on partitions (3 tiles of 128), spatial flattened in free dim
  - x is uploaded as fp16 and DMA'd straight into the dx=0 pool0 plane,
    so k=0 bins are exactly fp16(x); loads go on the sync queue AHEAD of
    the stores (measured ~4 us earlier start than the gpsimd queue)
  - for each (pool k, dx) a column-pre-shifted, row-replicated-padded plane
    R[k][dxi][t]; every bin is then a CONTIGUOUS row-window of one plane.
    Stores are grouped per (ctile, dy) only — never across ctiles — so the
    store queue is unblocked the moment each ctile's planes finish
  - dx!=0 planes are flat-shifted whole-plane fp16 copies (column shift ==
    element offset in the flat layout; row-boundary wrap errors land
    exactly in the clamped columns, fixed by strided edge-column copies)
  - pool1 separable 3x3 SUM runs in fp16 (2x DVE rate; |sums| <= ~20 so
    fp16 range is safe and the ~6 roundings keep abs err at a few e-3);
    count_include_pad=False normalization factorizes (cnt =
    rowcnt[i]*colcnt[j], each in {2,3}), so ONE tensor_scalar mul by 1/9
    normalizes the plane and four tiny edge-region muls by 1.5 fix the
    border rows/columns
  - DVE does almost everything (fp16 copies measured at ~0.3 ns/elem);
    the Act engine only takes one big plane copy per phase (it runs fp16
    at ~1 ns/elem with high small-op overhead)
  - no stride-0 (broadcast) APs, no GpSimd data ops (both measured slow)
"""

import numpy as np

B = 8
W = 37          # spatial side
WW = W * W      # 1369
D = 384
P = 128
ST = D // P     # 3 channel tiles of 128
NBINS = 29
NWR = 17        # bins actually written (the rest are zero)
PAD0, PAD1 = 1, 3
R0ROWS = W + 2 * PAD0             # 39
R1ROWS = W + 2 * PAD1             # 43
R0F = R0ROWS * W                  # 1443 flat elems per plane
R1F = R1ROWS * W                  # 1591

_CACHE = {}


def _build_nc():
    import concourse.bass as bass  # noqa: F401
    import concourse.tile as tile
    from concourse import bacc, mybir
    from contextlib import ExitStack

    f16 = mybir.dt.float16
    nc = bacc.Bacc("TRN2", target_bir_lowering=False, debug=False)

    xt = nc.declare_dram_parameter("xt", [ST, P, WW], f16, isOutput=False)
    out = nc.declare_dram_parameter("out", [NWR, ST, P, WW], f16, isOutput=True)

    with tile.TileContext(nc) as tc, ExitStack() as ctx:
        perm = ctx.enter_context(tc.tile_pool(name="perm", bufs=1))
        tmp = ctx.enter_context(tc.tile_pool(name="tmp", bufs=2))

        # R0: [dxi, t, 39, 37] (pad 1, dx in {-1,0,+1})
        # R1: [dxi, t, 43, 37] (pad 3, dx in {-3,0,+3})
        R0 = perm.tile([P, 3, ST, R0ROWS, W], f16, name="R0")
        R1 = perm.tile([P, 3, ST, R1ROWS, W], f16, name="R1")

        # ---- load fp16 x into the dx=0 plane centers, on the store queue
        # but ahead of every store ----
        for t in range(ST):
            nc.sync.dma_start(
                R0[:, 1, t, PAD0 : PAD0 + W, :].rearrange("p a b -> p (a b)"),
                xt.ap()[t],
            )

        for t in range(ST):
            # pad rows of the dx=0 plane (replicate first/last x row)
            nc.vector.tensor_copy(R0[:, 1, t, 0, :], R0[:, 1, t, 1, :])
            nc.vector.tensor_copy(R0[:, 1, t, R0ROWS - 1, :], R0[:, 1, t, R0ROWS - 2, :])

            # ---- dx=+-1 planes: flat-shifted whole-plane copies + col fix
            # (pad rows propagate; wrap errors land in the clamped column) ----
            p0f = R0[:, 1, t].rearrange("p a b -> p (a b)")
            f = R0[:, 0, t].rearrange("p a b -> p (a b)")
            nc.scalar.copy(f[:, 1:R0F], p0f[:, 0 : R0F - 1])
            nc.vector.tensor_copy(R0[:, 0, t, :, 0], R0[:, 1, t, :, 0])
            f = R0[:, 2, t].rearrange("p a b -> p (a b)")
            nc.vector.tensor_copy(f[:, 0 : R0F - 1], p0f[:, 1:R0F])
            nc.vector.tensor_copy(R0[:, 2, t, :, W - 1], R0[:, 1, t, :, W - 1])

            # ---- k=0 stores for this ctile (one DMA per dy group) ----
            for r_i, dy in enumerate((-1, 0, 1)):
                src = R0[:, :, t, PAD0 + dy : PAD0 + dy + W, :].rearrange(
                    "p x a b -> p x (a b)"
                )
                dst = out.ap()[3 * r_i : 3 * r_i + 3, t].transpose([1, 0, 2])
                nc.sync.dma_start(dst, src)

        for t in range(ST):
            Xc = R0[:, 1, t, PAD0 : PAD0 + W, :].rearrange("p a b -> p (a b)")
            Xc3 = R0[:, 1, t, PAD0 : PAD0 + W, :]

            # ---- column pass (fp16 sums):
            # T[i,j] = sum_dx X[i, j+dx] (zero outside) ----
            T = tmp.tile([P, WW], f16, name="T", tag="T")
            T3 = T.rearrange("p (a b) -> p a b", a=W, b=W)
            nc.vector.tensor_add(T[:, 0 : WW - 1], Xc[:, 0 : WW - 1], Xc[:, 1:WW])
            nc.vector.tensor_copy(T[:, WW - 1 : WW], Xc[:, WW - 1 : WW])
            nc.vector.tensor_add(T[:, 1:WW], T[:, 1:WW], Xc[:, 0 : WW - 1])
            nc.vector.tensor_add(T3[:, :, 0], Xc3[:, :, 0], Xc3[:, :, 1])
            nc.vector.tensor_add(T3[:, :, W - 1], Xc3[:, :, W - 2], Xc3[:, :, W - 1])

            # ---- row pass into a padded fp16 SUM plane ----
            Sp = tmp.tile([P, R1F], f16, name="Sp", tag="Sp")
            c0 = PAD1 * W                        # 111: center start
            nW = WW - W
            nc.vector.tensor_add(Sp[:, c0 : c0 + nW], T[:, 0:nW], T[:, W:WW])
            nc.vector.tensor_copy(Sp[:, c0 + nW : c0 + WW], T[:, nW:WW])
            nc.vector.tensor_add(Sp[:, c0 + W : c0 + WW], Sp[:, c0 + W : c0 + WW], T[:, 0:nW])
            # pad rows: replicate first/last center row (contiguous copies)
            for i in range(PAD1):
                nc.vector.tensor_copy(Sp[:, i * W : (i + 1) * W], Sp[:, c0 : c0 + W])
                nc.vector.tensor_copy(
                    Sp[:, (PAD1 + W + i) * W : (PAD1 + W + i + 1) * W],
                    Sp[:, (PAD1 + W - 1) * W : (PAD1 + W) * W],
                )

            # ---- normalize in ONE pass: interior count is 9; border
            # rows/cols have count 2 (not 3) per axis -> x1.5 ----
            Pc = R1[:, 1, t]
            Pcf = Pc.rearrange("p a b -> p (a b)")
            nc.vector.tensor_scalar_mul(Pcf[:, :], Sp[:, :], 1.0 / 9.0)
            nc.vector.tensor_scalar_mul(Pcf[:, 0 : c0 + W], Pcf[:, 0 : c0 + W], 1.5)
            nc.vector.tensor_scalar_mul(
                Pcf[:, R1F - c0 - W : R1F], Pcf[:, R1F - c0 - W : R1F], 1.5
            )
            nc.vector.tensor_scalar_mul(Pc[:, :, 0], Pc[:, :, 0], 1.5)
            nc.vector.tensor_scalar_mul(Pc[:, :, W - 1], Pc[:, :, W - 1], 1.5)

            # ---- dx=+-3 planes: flat-shifted fp16 copies + edge-col fixes ----
            f = R1[:, 0, t].rearrange("p a b -> p (a b)")
            nc.scalar.copy(f[:, 3:R1F], Pcf[:, 0 : R1F - 3])
            f = R1[:, 2, t].rearrange("p a b -> p (a b)")
            nc.vector.tensor_copy(f[:, 0 : R1F - 3], Pcf[:, 3:R1F])
            for c in range(PAD1):
                nc.vector.tensor_copy(R1[:, 0, t, :, c], Pc[:, :, 0])
                nc.vector.tensor_copy(R1[:, 2, t, :, W - 1 - c], Pc[:, :, W - 1])

            # ---- k=1 stores for this ctile (per dy group) ----
            for dy, dxis, p0 in ((-3, None, 9), (0, 0, 12), (0, 2, 13), (3, None, 14)):
                lo = PAD1 + dy
                if dxis is None:
                    src = R1[:, :, t, lo : lo + W, :].rearrange("p x a b -> p x (a b)")
                    dst = out.ap()[p0 : p0 + 3, t].transpose([1, 0, 2])
                else:
                    src = R1[:, dxis, t, lo : lo + W, :].rearrange("p a b -> p (a b)")
                    dst = out.ap()[p0, t]
                nc.sync.dma_start(dst, src)

    nc.compile()
    return nc


def get_nc():
    if "nc" not in _CACHE:
        _CACHE["nc"] = _build_nc()
    return _CACHE["nc"]


def make_in_maps(x: np.ndarray):
    x = np.ascontiguousarray(x, dtype=np.float32)
    assert x.shape == (B, W, W, D), x.shape
    maps = []
    for b in range(B):
        xtr = x[b].transpose(2, 0, 1).reshape(ST, P, WW).astype(np.float16)
        maps.append({"xt": np.ascontiguousarray(xtr)})
    return maps


def run(x: np.ndarray, **kw):
    from concourse.bass_utils import run_bass_kernel_spmd

    nc = get_nc()
    res = run_bass_kernel_spmd(nc, make_in_maps(x), core_ids=list(range(B)), **kw)
    outs = np.zeros((B, NBINS * D, W, W), np.float32)
    for b in range(B):
        ob = np.asarray(res.results[b]["out"], dtype=np.float32)
        outs[b, : NWR * D] = ob.reshape(NWR * D, W, W)
    return outs, res


def kernel(x: np.ndarray) -> np.ndarray:
    outs, _ = run(x)
    return outs


# revision 4
# speedup vs baseline: 1.1572x; 1.1371x over previous
"""DINOv2 LoRA featurizer histogram-binning kernel for TRN2 (8 NeuronCores).

Reference computation (per sample):
  x: [37, 37, 384] -> bx = x^T [384, 37, 37]
  pool0 = bx, pool1 = AvgPool2d(3, stride 1, pad 1, count_include_pad=False)
  17 bins = border-clamped shifts of pool0 (9 bins, offsets +-1) and
  pool1 (8 bins, offsets +-3); bins 17..28 of 29 are zero.
  out = [29*384, 37, 37] with channel c = bin*384 + feature.

Sharding: pure data parallel, sample b -> core b (B == 8 == n_cores).

The kernel is store-bandwidth bound (17 bins x 2.1 MB per core). This
version halves the store stream by writing fp16 (rel err ~4e-4, far
inside the 2e-2 gate) and keeps only the 17 real bins in DRAM; the 12
zero bins are assembled host-side as zeros.

Device strategy (per core), built so the store-DMA stream (~18 MB fp16)
is the only critical path:
  - channels on partitions (3 tiles of 128), spatial flattened in free dim
  - x is uploaded as fp16 and DMA'd straight into the dx=0 pool0 plane,
    so k=0 bins are exactly fp16(x); loads go on the sync queue AHEAD of
    the stores (measured ~4 us earlier start than the gpsimd queue)
  - for each (pool k, dx) a column-pre-shifted, row-replicated-padded plane
    R[k][dxi][t]; every bin is then a CONTIGUOUS row-window of one plane.
    Stores are grouped per (ctile, dy) only — never across ctiles — so the
    store queue is unblocked the moment each ctile's planes finish
  - dx!=0 planes are flat-shifted whole-plane fp16 copies (column shift ==
    element offset in the flat layout; row-boundary wrap errors land
    exactly in the clamped columns, fixed by strided edge-column copies)
  - pool1 separable 3x3 SUM runs in fp16 (2x DVE rate; |sums| <= ~20 so
    fp16 range is safe and the ~6 roundings keep abs err at a few e-3);
    count_include_pad=False normalization factorizes (cnt =
    rowcnt[i]*colcnt[j], each in {2,3}), so ONE tensor_scalar mul by 1/9
    normalizes the plane and four tiny edge-region muls by 1.5 fix the
    border rows/columns
  - DVE does almost everything (fp16 copies measured at ~0.3 ns/elem);
    the Act engine only takes one big plane copy per phase (it runs fp16
    at ~1 ns/elem with high small-op overhead)
  - no stride-0 (broadcast) APs, no GpSimd data ops (both measured slow)
"""

import numpy as np

B = 8
W = 37          # spatial side
WW = W * W      # 1369
D = 384
P = 128
ST = D // P     # 3 channel tiles of 128
NBINS = 29
NWR = 17        # bins actually written (the rest are zero)
PAD0, PAD1 = 1, 3
R0ROWS = W + 2 * PAD0             # 39
R1ROWS = W + 2 * PAD1             # 43
R0F = R0ROWS * W                  # 1443 flat elems per plane
R1F = R1ROWS * W                  # 1591

_CACHE = {}


def _build_nc():
    import concourse.bass as bass  # noqa: F401
    import concourse.tile as tile
    from concourse import bacc, mybir
    from contextlib import ExitStack

    f16 = mybir.dt.float16
    nc = bacc.Bacc("TRN2", target_bir_lowering=False, debug=False)

    xt = nc.declare_dram_parameter("xt", [ST, P, WW], f16, isOutput=False)
    out = nc.declare_dram_parameter("out", [ST, P, NWR, WW], f16, isOutput=True)

    with tile.TileContext(nc) as tc, ExitStack() as ctx:
        perm = ctx.enter_context(tc.tile_pool(name="perm", bufs=1))
        tmp = ctx.enter_context(tc.tile_pool(name="tmp", bufs=2))

        # R0: [dxi, t, 39, 37] (pad 1, dx in {-1,0,+1})
        # R1: [dxi, t, 43, 37] (pad 3, dx in {-3,0,+3})
        R0 = perm.tile([P, 3, ST, R0ROWS, W], f16, name="R0")
        R1 = perm.tile([P, 3, ST, R1ROWS, W], f16, name="R1")

        # ---- load fp16 x into the dx=0 plane centers, on the store queue
        # but ahead of every store ----
        for t in range(ST):
            nc.sync.dma_start(
                R0[:, 1, t, PAD0 : PAD0 + W, :].rearrange("p a b -> p (a b)"),
                xt.ap()[t],
            )

        for t in range(ST):
            # pad rows of the dx=0 plane (replicate first/last x row)
            nc.vector.tensor_copy(R0[:, 1, t, 0, :], R0[:, 1, t, 1, :])
            nc.vector.tensor_copy(R0[:, 1, t, R0ROWS - 1, :], R0[:, 1, t, R0ROWS - 2, :])

            # ---- dx=+-1 planes: flat-shifted whole-plane copies + col fix
            # (pad rows propagate; wrap errors land in the clamped column) ----
            p0f = R0[:, 1, t].rearrange("p a b -> p (a b)")
            f = R0[:, 0, t].rearrange("p a b -> p (a b)")
            nc.vector.tensor_copy(f[:, 1:R0F], p0f[:, 0 : R0F - 1])
            nc.vector.tensor_copy(R0[:, 0, t, :, 0], R0[:, 1, t, :, 0])
            f = R0[:, 2, t].rearrange("p a b -> p (a b)")
            nc.vector.tensor_copy(f[:, 0 : R0F - 1], p0f[:, 1:R0F])
            nc.vector.tensor_copy(R0[:, 2, t, :, W - 1], R0[:, 1, t, :, W - 1])

            # ---- k=0 stores for this ctile (one DMA per dy group) ----
            for r_i, dy in enumerate((-1, 0, 1)):
                src = R0[:, :, t, PAD0 + dy : PAD0 + dy + W, :].rearrange(
                    "p x a b -> p x (a b)"
                )
                dst = out.ap()[t][:, 3 * r_i : 3 * r_i + 3, :]
                nc.sync.dma_start(dst, src)

        for t in range(ST):
            Xc = R0[:, 1, t, PAD0 : PAD0 + W, :].rearrange("p a b -> p (a b)")
            Xc3 = R0[:, 1, t, PAD0 : PAD0 + W, :]

            # ---- column pass (fp16 sums):
            # T[i,j] = sum_dx X[i, j+dx] (zero outside) ----
            T = tmp.tile([P, WW], f16, name="T", tag="T")
            T3 = T.rearrange("p (a b) -> p a b", a=W, b=W)
            nc.vector.tensor_add(T[:, 0 : WW - 1], Xc[:, 0 : WW - 1], Xc[:, 1:WW])
            nc.vector.tensor_copy(T[:, WW - 1 : WW], Xc[:, WW - 1 : WW])
            nc.vector.tensor_add(T[:, 1:WW], T[:, 1:WW], Xc[:, 0 : WW - 1])
            nc.vector.tensor_add(T3[:, :, 0], Xc3[:, :, 0], Xc3[:, :, 1])
            nc.vector.tensor_add(T3[:, :, W - 1], Xc3[:, :, W - 2], Xc3[:, :, W - 1])

            # ---- row pass into a padded fp16 SUM plane ----
            Sp = tmp.tile([P, R1F], f16, name="Sp", tag="Sp")
            c0 = PAD1 * W                        # 111: center start
            nW = WW - W
            nc.vector.tensor_add(Sp[:, c0 : c0 + nW], T[:, 0:nW], T[:, W:WW])
            nc.vector.tensor_copy(Sp[:, c0 + nW : c0 + WW], T[:, nW:WW])
            nc.vector.tensor_add(Sp[:, c0 + W : c0 + WW], Sp[:, c0 + W : c0 + WW], T[:, 0:nW])
            # pad rows: replicate first/last center row (contiguous copies)
            for i in range(PAD1):
                nc.vector.tensor_copy(Sp[:, i * W : (i + 1) * W], Sp[:, c0 : c0 + W])
                nc.vector.tensor_copy(
                    Sp[:, (PAD1 + W + i) * W : (PAD1 + W + i + 1) * W],
                    Sp[:, (PAD1 + W - 1) * W : (PAD1 + W) * W],
                )

            # ---- normalize in ONE pass: interior count is 9; border
            # rows/cols have count 2 (not 3) per axis -> x1.5 ----
            Pc = R1[:, 1, t]
            Pcf = Pc.rearrange("p a b -> p (a b)")
            nc.vector.tensor_scalar_mul(Pcf[:, :], Sp[:, :], 1.0 / 9.0)
            nc.vector.tensor_scalar_mul(Pcf[:, 0 : c0 + W], Pcf[:, 0 : c0 + W], 1.5)
            nc.vector.tensor_scalar_mul(
                Pcf[:, R1F - c0 - W : R1F], Pcf[:, R1F - c0 - W : R1F], 1.5
            )
            nc.vector.tensor_scalar_mul(Pc[:, :, 0], Pc[:, :, 0], 1.5)
            nc.vector.tensor_scalar_mul(Pc[:, :, W - 1], Pc[:, :, W - 1], 1.5)

            # ---- dx=+-3 planes: flat-shifted fp16 copies + edge-col fixes ----
            f = R1[:, 0, t].rearrange("p a b -> p (a b)")
            nc.vector.tensor_copy(f[:, 3:R1F], Pcf[:, 0 : R1F - 3])
            f = R1[:, 2, t].rearrange("p a b -> p (a b)")
            nc.vector.tensor_copy(f[:, 0 : R1F - 3], Pcf[:, 3:R1F])
            for c in range(PAD1):
                nc.vector.tensor_copy(R1[:, 0, t, :, c], Pc[:, :, 0])
                nc.vector.tensor_copy(R1[:, 2, t, :, W - 1 - c], Pc[:, :, W - 1])

            # ---- k=1 stores for this ctile: dy=+-3 as 3-bin dy groups
            # (contiguous 8 KiB DRAM runs per partition); the two dy=0 bins
            # (12: dx=-3, 13: dx=+3) are staged into one contiguous SBUF
            # pair so their store also writes adjacent bins ----
            S = tmp.tile([P, 2, WW], f16, name="S", tag="S")
            nc.vector.tensor_copy(
                S[:, 0, :], R1[:, 0, t, PAD1 : PAD1 + W, :].rearrange("p a b -> p (a b)")
            )
            nc.vector.tensor_copy(
                S[:, 1, :], R1[:, 2, t, PAD1 : PAD1 + W, :].rearrange("p a b -> p (a b)")
            )
            for dy, p0 in ((-3, 9), (3, 14)):
                lo = PAD1 + dy
                src = R1[:, :, t, lo : lo + W, :].rearrange("p x a b -> p x (a b)")
                nc.sync.dma_start(out.ap()[t][:, p0 : p0 + 3, :], src)
            nc.sync.dma_start(out.ap()[t][:, 12:14, :], S[:, :, :])

    nc.compile()
    return nc


def get_nc():
    if "nc" not in _CACHE:
        _CACHE["nc"] = _build_nc()
    return _CACHE["nc"]


def make_in_maps(x: np.ndarray):
    x = np.ascontiguousarray(x, dtype=np.float32)
    assert x.shape == (B, W, W, D), x.shape
    maps = []
    for b in range(B):
        xtr = x[b].transpose(2, 0, 1).reshape(ST, P, WW).astype(np.float16)
        maps.append({"xt": np.ascontiguousarray(xtr)})
    return maps


def run(x: np.ndarray, **kw):
    from concourse.bass_utils import run_bass_kernel_spmd

    nc = get_nc()
    res = run_bass_kernel_spmd(nc, make_in_maps(x), core_ids=list(range(B)), **kw)
    outs = np.zeros((B, NBINS * D, W, W), np.float32)
    for b in range(B):
        ob = np.asarray(res.results[b]["out"], dtype=np.float32)
        ob = ob.reshape(D, NWR, W, W).transpose(1, 0, 2, 3)
        outs[b, : NWR * D] = ob.reshape(NWR * D, W, W)
    return outs, res


def kernel(x: np.ndarray) -> np.ndarray:
    outs, _ = run(x)
    return outs
